# revision 10
# baseline (speedup 1.0000x reference)
"""Trainium2 Bass kernel for a spiking (LIF) recurrent network.

Reference semantics (per timestep t):
    inp_h = x_t @ w_ih + s_h @ w_hh          # s_h = previous step's hidden spikes
    s_h, v_h, r_h = lif(v_h, r_h, inp_h)
    inp_o = s_h @ w_ho
    s_o, v_o, r_o = lif(v_o, r_o, inp_o)
    y[t] = s_o

Sharding: data-parallel over batch (256 -> 32 per core across 8 cores),
weights replicated, time recurrence local per core.

On-chip layout "L": hidden state [B=32, NH=2048] is stored as a [128, 512]
tile with partition p = q*32 + b (q = hidden quarter, b = batch) and free
column c = hidden index within quarter.  This gives full 128-partition
utilization for both the vector engine (LIF) and the tensor engine (the
w_hh matmul is issued as 4 column-group-tiled matmuls per k-tile, one per
hidden quarter, all sharing the same [128,32] transposed-spike stationary).
"""

import numpy as np

T_STEPS = 100
B_FULL = 256
N_CORES = 8
BL = B_FULL // N_CORES  # 32 batch per core
N_IN = 784
N_HID = 2048
N_OUT = 10
BIG = float(2.0**19)

_COMPILED = {}


def _build(t_steps):
    import concourse.mybir as mybir
    import concourse.tile as tile
    from concourse import bacc

    F32 = mybir.dt.float32
    Alu = mybir.AluOpType
    Act = mybir.ActivationFunctionType

    T = t_steps
    assert T % 4 == 0 and T >= 8
    MT = T * BL // 128  # number of 128-row m-tiles in the X precompute
    KI = 7  # 784 = 7 * 112 input k-tiles
    KH = 16  # 2048 = 16 * 128 hidden k-tiles

    nc = bacc.Bacc("TRN2", target_bir_lowering=False, debug=False,
                   num_devices=N_CORES)

    # const AP for the Relu bias value used by the LIF refractory decrement
    _c = nc.alloc_sbuf_tensor("const-f32-m0p5", [128, 1], F32)
    nc.gpsimd.memset(_c.ap(), -0.5)
    nc.const_aps.aps[(F32, -0.5)] = _c.ap()
    nc.all_engine_barrier()

    x_d = nc.dram_tensor("x", [T, BL, N_IN], F32, kind="ExternalInput")
    wih_d = nc.dram_tensor("wih", [N_IN, N_HID], F32, kind="ExternalInput")
    whh_d = nc.dram_tensor("whh", [N_HID, N_HID], F32, kind="ExternalInput")
    who_d = nc.dram_tensor("who", [N_HID, N_OUT], F32, kind="ExternalInput")
    ident_d = nc.dram_tensor("ident", [128, 32], F32, kind="ExternalInput")
    i128_d = nc.dram_tensor("i128", [128, 128], F32, kind="ExternalInput")
    y_d = nc.dram_tensor("y", [BL, T * N_OUT], F32, kind="ExternalOutput")
    xpre_d = nc.dram_tensor("xpre", [T * BL, N_HID], F32)  # internal scratch

    with tile.TileContext(nc) as tc:
        with (
            tc.tile_pool(name="const", bufs=1) as cpool,
            tc.tile_pool(name="weights", bufs=1) as wpool,
            tc.tile_pool(name="xt", bufs=9) as xtpool,
            tc.tile_pool(name="stage", bufs=3) as stpool,
            tc.tile_pool(name="state", bufs=1) as vpool,
            tc.tile_pool(name="tmp", bufs=1) as tpool,
            tc.tile_pool(name="tmp2", bufs=2) as t2pool,
            tc.tile_pool(name="xin", bufs=2) as xpool,
            tc.tile_pool(name="psum", bufs=2, space="PSUM") as ppool,
            tc.tile_pool(name="psum4", bufs=4, space="PSUM") as ppool4,
        ):
            # ---- constants ----
            ident_sb = cpool.tile([128, 32], F32, tag="ident")
            nc.sync.dma_start(ident_sb[:], ident_d.ap())
            zrow = cpool.tile([1, 512], F32, tag="zrow")
            nc.vector.memset(zrow[:], 0.0)
            i128_sb = cpool.tile([128, 128], F32, tag="i128")
            nc.sync.dma_start(i128_sb[:], i128_d.ap())
            who_sb = cpool.tile([128, KH * N_OUT], F32, tag="who")
            nc.sync.dma_start(
                who_sb[:].rearrange("p (kt o) -> p kt o", kt=KH),
                who_d.ap().rearrange("(kt p) o -> p kt o", p=128),
            )

            # ---- phase 1: X = x @ w_ih, in two column halves ----
            wih_sb = wpool.tile([112, KI * 1024], F32, tag="wih")
            for h in range(2):
                for k in range(KI):
                    nc.sync.dma_start(
                        wih_sb[:, k * 1024:(k + 1) * 1024],
                        wih_d.ap()[k * 112:(k + 1) * 112,
                                   h * 1024:(h + 1) * 1024],
                    )
                for m in range(MT):
                    xts = []
                    for k in range(KI):
                        xt = xtpool.tile([112, 128], F32, tag="xt")
                        nc.sync.dma_start(
                            xt[:].rearrange("p (t b) -> p t b", t=4),
                            x_d.ap()[4 * m:4 * m + 4, :,
                                     112 * k:112 * (k + 1)]
                            .rearrange("t b k -> k t b"),
                        )
                        xts.append(xt)
                    for qh in range(2):
                        q = 2 * h + qh
                        pp = ppool.tile([128, 512], F32, tag="ph")
                        for k in range(KI):
                            nc.tensor.matmul(
                                pp[:],
                                xts[k][:],
                                wih_sb[:, k * 1024 + qh * 512:
                                       k * 1024 + qh * 512 + 512],
                                start=(k == 0), stop=(k == KI - 1),
                            )
                        stg = stpool.tile([128, 512], F32, tag="stage")
                        nc.scalar.copy(stg[:], pp[:])
                        nc.sync.dma_start(
                            xpre_d.ap()[m * 128:(m + 1) * 128,
                                        q * 512:(q + 1) * 512],
                            stg[:],
                        )

            # ---- load w_hh (after phase 1 so SBUF peak stays low) ----
            whh_sb = wpool.tile([128, KH * N_HID], F32, tag="whh")
            for kt in range(KH):
                nc.sync.dma_start(
                    whh_sb[:, kt * N_HID:(kt + 1) * N_HID],
                    whh_d.ap()[kt * 128:(kt + 1) * 128, :],
                )

            # ---- phase 2: the recurrence ----
            v = vpool.tile([128, 512], F32, tag="v")
            r = vpool.tile([128, 512], F32, tag="r")
            vo = vpool.tile([BL, N_OUT], F32, tag="vo")
            ro = vpool.tile([BL, N_OUT], F32, tag="ro")
            y_acc = vpool.tile([BL, T * N_OUT], F32, tag="yacc")
            nc.vector.memset(v[:], 0.0)
            nc.vector.memset(r[:], 0.0)
            nc.vector.memset(vo[:], 0.0)
            nc.vector.memset(ro[:], 0.0)

            def out_lif(po, tau):
                """Output-layer LIF for output step tau, input in psum po."""
                nvdec = tpool.tile([BL, N_OUT], F32, tag="onvdec")
                nc.vector.scalar_tensor_tensor(
                    nvdec[:], vo[:], 0.1, vo[:], Alu.mult, Alu.subtract)
                rdec = tpool.tile([BL, N_OUT], F32, tag="ordec")
                nc.scalar.activation(rdec[:], ro[:], Act.Relu, bias=-0.5)
                nthr = tpool.tile([BL, N_OUT], F32, tag="onthr")
                nc.scalar.activation(nthr[:], rdec[:], Act.Copy,
                                     scale=-BIG, bias=-1.0)
                s = y_acc[:, tau * N_OUT:(tau + 1) * N_OUT]
                nc.vector.tensor_tensor(s, nvdec[:], nthr[:], Alu.is_le)
                nc.vector.tensor_tensor(ro[:], rdec[:], s, Alu.add)
                g = tpool.tile([BL, N_OUT], F32, tag="og")
                nc.scalar.activation(g[:], ro[:], Act.Relu,
                                     scale=-2.0, bias=1.0)
                gi = tpool.tile([BL, N_OUT], F32, tag="ogi")
                nc.vector.tensor_tensor(gi[:], g[:], po[:BL, :], Alu.mult)
                u2 = tpool.tile([BL, N_OUT], F32, tag="ou2")
                nc.vector.tensor_tensor(u2[:], gi[:], nvdec[:], Alu.subtract)
                sb = tpool.tile([BL, N_OUT], F32, tag="osb")
                nc.vector.tensor_scalar(sb[:], ro[:], 0.5, None, Alu.is_le)
                nc.vector.tensor_tensor(vo[:], sb[:], u2[:], Alu.mult)

            st_prev = None
            for t in range(T):
                # X(t) tile in layout L
                xt_t = xpool.tile([128, 512], F32, tag="X")
                for q in range(4):
                    nc.sync.dma_start(
                        xt_t[32 * q:32 * q + 32, :],
                        xpre_d.ap()[t * BL:(t + 1) * BL,
                                    q * 512:(q + 1) * 512],
                    )
                # inp_h accumulation: inject X, then w_hh partials
                ph = ppool.tile([128, 512], F32, tag="ph")
                nc.tensor.matmul(ph[:], i128_sb[:], xt_t[:],
                                 start=True, stop=False)
                if t > 0:
                    for kt in range(KH):
                        stat = st_prev[:, 32 * kt:32 * kt + 32]
                        for q in range(4):
                            nc.tensor.matmul(
                                ph[32 * q:32 * q + 32, :],
                                stat,
                                whh_sb[:, kt * N_HID + q * 512:
                                       kt * N_HID + q * 512 + 512],
                                start=False, stop=False,
                                tile_position=(0, 32 * q),
                            )
                # closing zero-matmul: covers all 128 partitions so the
                # accumulation group's stop flag is partition-uniform
                nc.tensor.matmul(ph[:], zrow[:1, 0:128], zrow[:1, 0:512],
                                 start=False, stop=True)
                if t > 0:
                    po = ppool.tile([BL, N_OUT], F32, tag="po")
                    for kt in range(KH):
                        nc.tensor.matmul(
                            po[:],
                            st_prev[:, 32 * kt:32 * kt + 32],
                            who_sb[:, kt * N_OUT:(kt + 1) * N_OUT],
                            start=(kt == 0), stop=(kt == KH - 1),
                        )

                # hidden LIF (exact fp32 rounding match with the reference)
                nvdec = t2pool.tile([128, 512], F32, tag="nvdec")
                nc.vector.scalar_tensor_tensor(
                    nvdec[:], v[:], 0.1, v[:], Alu.mult, Alu.subtract)
                rdec = tpool.tile([128, 512], F32, tag="rdec")
                nc.scalar.activation(rdec[:], r[:], Act.Relu, bias=-0.5)
                nthr = tpool.tile([128, 512], F32, tag="nthr")
                nc.scalar.activation(nthr[:], rdec[:], Act.Copy,
                                     scale=-BIG, bias=-1.0)
                sl = t2pool.tile([128, 512], F32, tag="sl")
                nc.vector.tensor_tensor(sl[:], nvdec[:], nthr[:], Alu.is_le)
                nc.vector.tensor_tensor(r[:], rdec[:], sl[:], Alu.add)
                g = tpool.tile([128, 512], F32, tag="g")
                nc.scalar.activation(g[:], r[:], Act.Relu,
                                     scale=-2.0, bias=1.0)
                gi = tpool.tile([128, 512], F32, tag="gi")
                nc.vector.tensor_tensor(gi[:], g[:], ph[:], Alu.mult)
                u2 = tpool.tile([128, 512], F32, tag="u2")
                nc.vector.tensor_tensor(u2[:], gi[:], nvdec[:], Alu.subtract)
                sbar = tpool.tile([128, 512], F32, tag="sbar")
                nc.vector.tensor_scalar(sbar[:], r[:], 0.5, None, Alu.is_le)
                nc.vector.tensor_tensor(v[:], sbar[:], u2[:], Alu.mult)

                # transpose spikes for next step's stationaries.  Matmuls in
                # different PE row-groups run concurrently, so each row-group
                # must target its own PSUM bank (same-bank concurrent PE
                # writes wedge the device).
                st = t2pool.tile([128, 512], F32, tag="st")
                for qq in range(4):
                    psq = ppool4.tile([128, 128], F32, tag="ps")
                    for rr in range(4):
                        nc.tensor.matmul(
                            psq[:, 32 * rr:32 * rr + 32],
                            sl[32 * qq:32 * qq + 32, 128 * rr:128 * rr + 128],
                            ident_sb[32 * qq:32 * qq + 32, :],
                            start=(rr == 0), stop=(rr == 3),
                            tile_position=(32 * qq, 0),
                        )
                    nc.scalar.copy(st[:, 128 * qq:128 * (qq + 1)], psq[:])

                if t > 0:
                    out_lif(po, t - 1)
                st_prev = st

            # epilogue: output step T-1
            po = ppool.tile([BL, N_OUT], F32, tag="po")
            for kt in range(KH):
                nc.tensor.matmul(
                    po[:],
                    st_prev[:, 32 * kt:32 * kt + 32],
                    who_sb[:, kt * N_OUT:(kt + 1) * N_OUT],
                    start=(kt == 0), stop=(kt == KH - 1),
                )
            out_lif(po, T - 1)

            nc.sync.dma_start(y_d.ap(), y_acc[:])

    nc.compile()
    return nc


def _get_program(t_steps):
    if t_steps not in _COMPILED:
        _COMPILED[t_steps] = _build(t_steps)
    return _COMPILED[t_steps]


def _make_in_maps(x_in, w_ih, w_hh, w_ho, t_steps):
    ident = np.zeros((128, 32), np.float32)
    for q in range(4):
        ident[32 * q:32 * q + 32, :] += np.eye(32, dtype=np.float32)
    i128 = np.eye(128, dtype=np.float32)
    shared = {
        "wih": np.ascontiguousarray(w_ih, np.float32),
        "whh": np.ascontiguousarray(w_hh, np.float32),
        "who": np.ascontiguousarray(w_ho, np.float32),
        "ident": ident,
        "i128": i128,
    }
    in_maps = []
    for c in range(N_CORES):
        m = dict(shared)
        m["x"] = np.ascontiguousarray(
            x_in[:t_steps, c * BL:(c + 1) * BL, :], np.float32)
        in_maps.append(m)
    return in_maps


def _run(x_in, w_ih, w_hh, w_ho, t_steps, trace=False):
    from concourse.bass_utils import run_bass_kernel_spmd

    nc = _get_program(t_steps)
    in_maps = _make_in_maps(x_in, w_ih, w_hh, w_ho, t_steps)
    res = run_bass_kernel_spmd(nc, in_maps, core_ids=list(range(N_CORES)),
                               trace=trace)
    parts = []
    for c in range(N_CORES):
        yc = res.results[c]["y"].reshape(BL, t_steps, N_OUT)
        parts.append(np.transpose(yc, (1, 0, 2)))
    y = np.concatenate(parts, axis=1)  # [T, B, N_OUT]
    return y, res


def kernel(x_in, w_ih, w_hh, w_ho, time):
    t_steps = int(time)
    assert t_steps == T_STEPS, f"kernel hardcoded for T={T_STEPS}, got {t_steps}"
    y, _ = _run(np.asarray(x_in), np.asarray(w_ih), np.asarray(w_hh),
                np.asarray(w_ho), t_steps)
    return y


# revision 16
# speedup vs baseline: 22.1897x; 22.1897x over previous
"""Trainium2 Bass kernel for a spiking (LIF) recurrent network.

Reference semantics (per timestep t):
    inp_h = x_t @ w_ih + s_h @ w_hh          # s_h = previous step's hidden spikes
    s_h, v_h, r_h = lif(v_h, r_h, inp_h)
    inp_o = s_h @ w_ho
    s_o, v_o, r_o = lif(v_o, r_o, inp_o)
    y[t] = s_o

Sharding: data-parallel over batch (256 -> 32 per core across 8 cores),
weights replicated, time recurrence local per core.

On-chip layout "L": hidden state [B=32, NH=2048] is stored as a [128, 512]
tile with partition p = q*32 + b (q = hidden quarter, b = batch) and free
column c = hidden index within quarter.  This gives full 128-partition
utilization for both the vector engine (LIF) and the tensor engine (the
w_hh matmul is issued as 4 column-group-tiled matmuls per k-tile, one per
hidden quarter, all sharing the same [128,32] transposed-spike stationary).

The LIF update is restructured so the next step's spikes come straight off
the PSUM in four chained vector ops ("early spike"), keeping the PE fed:
    u2   = g*inp - nvdec          # = vdec + g*inp   (nvdec = -vdec)
    nu   = fl(0.1*u2) - u2        # = -decay(v_new), exact reference rounding
    s'   = (nu <= nthr')          # spike test for the NEXT step
    nvdec'= sbar * nu             # -vdec for the next step (0 if spiked)
All comparisons/gates are exact in fp32 (thresholds use BIG=2^19), so the
kernel is bit-exact against the jax reference.
"""

import numpy as np

T_STEPS = 100
B_FULL = 256
N_CORES = 8
BL = B_FULL // N_CORES  # 32 batch per core
N_IN = 784
N_HID = 2048
N_OUT = 10
BIG = float(2.0**19)

_COMPILED = {}


def _build(t_steps):
    import concourse.mybir as mybir
    import concourse.tile as tile
    from concourse import bacc

    F32 = mybir.dt.float32
    Alu = mybir.AluOpType
    Act = mybir.ActivationFunctionType

    T = t_steps
    assert T % 4 == 0 and T >= 8
    MT = T * BL // 128  # number of 128-row m-tiles in the X precompute
    KI = 7  # 784 = 7 * 112 input k-tiles
    KH = 16  # 2048 = 16 * 128 hidden k-tiles

    nc = bacc.Bacc("TRN2", target_bir_lowering=False, debug=False,
                   num_devices=N_CORES)

    # const AP for the Relu bias value used by the LIF refractory decrement
    _c = nc.alloc_sbuf_tensor("const-f32-m0p5", [128, 1], F32)
    nc.gpsimd.memset(_c.ap(), -0.5)
    nc.const_aps.aps[(F32, -0.5)] = _c.ap()
    nc.all_engine_barrier()

    # x is fed pre-transposed from the host: [MT, KI, 112, 128] where
    # xth[m,k,kk,tl*32+b] = x[4m+tl, b, 112k+kk]  (contiguous DMA tiles)
    xth_d = nc.dram_tensor("xth", [MT, KI, 112, 128], F32, kind="ExternalInput")
    wih_d = nc.dram_tensor("wih", [N_IN, N_HID], F32, kind="ExternalInput")
    whh_d = nc.dram_tensor("whh", [N_HID, N_HID], F32, kind="ExternalInput")
    who_d = nc.dram_tensor("who", [N_HID, N_OUT], F32, kind="ExternalInput")
    ident_d = nc.dram_tensor("ident", [128, 32], F32, kind="ExternalInput")
    i128_d = nc.dram_tensor("i128", [128, 128], F32, kind="ExternalInput")
    y_d = nc.dram_tensor("y", [BL, T * N_OUT], F32, kind="ExternalOutput")
    xpre_d = nc.dram_tensor("xpre", [T * BL, N_HID], F32)  # internal scratch

    with tile.TileContext(nc) as tc:
        with (
            tc.tile_pool(name="const", bufs=1) as cpool,
            tc.tile_pool(name="weights", bufs=1) as wpool,
            tc.tile_pool(name="xt", bufs=8) as xtpool,
            tc.tile_pool(name="stage", bufs=2) as stpool,
            tc.tile_pool(name="state", bufs=1) as vpool,
            tc.tile_pool(name="tmp", bufs=2) as tpool,
            tc.tile_pool(name="tmp1", bufs=1) as t1pool,
            tc.tile_pool(name="xin", bufs=2) as xpool,
            tc.tile_pool(name="st3", bufs=2) as s3pool,
            tc.tile_pool(name="psum", bufs=2, space="PSUM") as ppool,
            tc.tile_pool(name="psum4", bufs=4, space="PSUM") as ppool4,
        ):
            # ---- constants ----
            ident_sb = cpool.tile([128, 32], F32, tag="ident")
            nc.sync.dma_start(ident_sb[:], ident_d.ap())
            zrow = cpool.tile([1, 160], F32, tag="zrow")
            nc.vector.memset(zrow[:], 0.0)
            i128_sb = cpool.tile([128, 128], F32, tag="i128")
            nc.sync.dma_start(i128_sb[:], i128_d.ap())
            who_sb = cpool.tile([128, KH * N_OUT], F32, tag="who")
            nc.sync.dma_start(
                who_sb[:].rearrange("p (kt o) -> p kt o", kt=KH),
                who_d.ap().rearrange("(kt p) o -> p kt o", p=128),
            )

            # ---- phase 1: X = x @ w_ih, in two column halves ----
            wih_sb = wpool.tile([112, KI * 1024], F32, tag="wih")
            for h in range(2):
                for k in range(KI):
                    nc.sync.dma_start(
                        wih_sb[:, k * 1024:(k + 1) * 1024],
                        wih_d.ap()[k * 112:(k + 1) * 112,
                                   h * 1024:(h + 1) * 1024],
                    )
                for m in range(MT):
                    xts = []
                    for k in range(KI):
                        xt = xtpool.tile([112, 128], F32, tag="xt")
                        nc.sync.dma_start(xt[:], xth_d.ap()[m, k])
                        xts.append(xt)
                    for qh in range(2):
                        q = 2 * h + qh
                        pp = ppool.tile([128, 512], F32, tag="ph")
                        for k in range(KI):
                            nc.tensor.matmul(
                                pp[:],
                                xts[k][:],
                                wih_sb[:, k * 1024 + qh * 512:
                                       k * 1024 + qh * 512 + 512],
                                start=(k == 0), stop=(k == KI - 1),
                            )
                        stg = stpool.tile([128, 512], F32, tag="stage")
                        nc.scalar.copy(stg[:], pp[:])
                        nc.sync.dma_start(
                            xpre_d.ap()[m * 128:(m + 1) * 128,
                                        q * 512:(q + 1) * 512],
                            stg[:],
                        )

            # ---- load w_hh (after phase 1 so SBUF peak stays low) ----
            whh_sb = wpool.tile([128, KH * N_HID], F32, tag="whh")
            for kt in range(KH):
                nc.sync.dma_start(
                    whh_sb[:, kt * N_HID:(kt + 1) * N_HID],
                    whh_d.ap()[kt * 128:(kt + 1) * 128, :],
                )

            # ---- phase 2: the recurrence ----
            # carried hidden state (rotating tiles)
            vo = vpool.tile([BL, N_OUT], F32, tag="vo")
            ro = vpool.tile([BL, N_OUT], F32, tag="ro")
            y_acc = vpool.tile([BL, T * N_OUT], F32, tag="yacc")
            nc.vector.memset(vo[:], 0.0)
            nc.vector.memset(ro[:], 0.0)

            sl = s3pool.tile([128, 512], F32, tag="sl")       # s_0 = 0
            nvdec = tpool.tile([128, 512], F32, tag="nvdec")  # -vdec_0 = 0
            rdec = tpool.tile([128, 512], F32, tag="rdec")    # rdec_0 = 0
            nthr = tpool.tile([128, 512], F32, tag="nthr")    # -(1+BIG*rdec_0)
            nc.vector.memset(sl[:], 0.0)
            nc.vector.memset(nvdec[:], 0.0)
            nc.vector.memset(rdec[:], 0.0)
            nc.vector.memset(nthr[:], -1.0)

            # psum_0 = X_0 (inject only; s_{-1} = 0)
            x_cur = xpool.tile([128, 512], F32, tag="X")
            for q in range(4):
                nc.sync.dma_start(
                    x_cur[32 * q:32 * q + 32, :],
                    xpre_d.ap()[0:BL, q * 512:(q + 1) * 512])
            ph_prev = ppool.tile([128, 512], F32, tag="ph")
            nc.tensor.matmul(ph_prev[:], i128_sb[:], x_cur[:],
                             start=True, stop=True)

            for i in range(T):
                # --- PE: transpose spikes s_i -> st_i (bank per row-group) ---
                st = s3pool.tile([128, 512], F32, tag="st")
                for qq in range(4):
                    psq = ppool4.tile([128, 128], F32, tag="ps")
                    for rr in range(4):
                        nc.tensor.matmul(
                            psq[:, 32 * rr:32 * rr + 32],
                            sl[32 * qq:32 * qq + 32, 128 * rr:128 * rr + 128],
                            ident_sb[32 * qq:32 * qq + 32, :],
                            start=(rr == 0), stop=(rr == 3),
                            tile_position=(32 * qq, 0),
                        )
                    nc.scalar.copy(st[:, 128 * qq:128 * (qq + 1)], psq[:])

                # --- PE: inp_o(i) = s_i @ w_ho ---
                po = ppool.tile([BL, N_OUT], F32, tag="po")
                for kt in range(KH):
                    nc.tensor.matmul(
                        po[:],
                        st[:, 32 * kt:32 * kt + 32],
                        who_sb[:, kt * N_OUT:(kt + 1) * N_OUT],
                        start=(kt == 0), stop=(kt == KH - 1),
                    )

                # --- PE: psum_{i+1} = X_{i+1} + s_i @ w_hh ---
                if i < T - 1:
                    x_nxt = xpool.tile([128, 512], F32, tag="X")
                    for q in range(4):
                        nc.sync.dma_start(
                            x_nxt[32 * q:32 * q + 32, :],
                            xpre_d.ap()[(i + 1) * BL:(i + 2) * BL,
                                        q * 512:(q + 1) * 512])
                    ph = ppool.tile([128, 512], F32, tag="ph")
                    nc.tensor.matmul(ph[:], i128_sb[:], x_nxt[:],
                                     start=True, stop=False)
                    for kt in range(KH):
                        stat = st[:, 32 * kt:32 * kt + 32]
                        for q in range(4):
                            nc.tensor.matmul(
                                ph[32 * q:32 * q + 32, :],
                                stat,
                                whh_sb[:, kt * N_HID + q * 512:
                                       kt * N_HID + q * 512 + 512],
                                start=False, stop=False,
                                tile_position=(0, 32 * q),
                            )
                    # closing zero-matmul: full-partition stop flag (sim's
                    # psum group tracker needs uniform partition coverage)
                    nc.tensor.matmul(ph[:, 0:1], zrow[:1, 0:128],
                                     zrow[:1, 0:1], start=False, stop=True)
                else:
                    ph = None

                # --- pre-psum LIF ops for step i (overlap the PE work) ---
                r_full = t1pool.tile([128, 512], F32, tag="rfull")
                nc.vector.tensor_tensor(r_full[:], rdec[:], sl[:], Alu.add)
                rdec_n = tpool.tile([128, 512], F32, tag="rdec")
                nc.scalar.activation(rdec_n[:], r_full[:], Act.Relu, bias=-0.5)
                nthr_n = tpool.tile([128, 512], F32, tag="nthr")
                nc.scalar.activation(nthr_n[:], rdec_n[:], Act.Copy,
                                     scale=-BIG, bias=-1.0)
                g = tpool.tile([128, 512], F32, tag="g")
                nc.scalar.activation(g[:], r_full[:], Act.Relu,
                                     scale=-2.0, bias=1.0)
                sbar = tpool.tile([128, 512], F32, tag="sbar")
                nc.vector.tensor_scalar(sbar[:], r_full[:], 0.5, None,
                                        Alu.is_le)

                # --- post-psum chain: consume psum_i, emit s_{i+1} ---
                gi = t1pool.tile([128, 512], F32, tag="gi")
                nc.vector.tensor_tensor(gi[:], g[:], ph_prev[:], Alu.mult)
                u2 = t1pool.tile([128, 512], F32, tag="u2")
                nc.vector.tensor_tensor(u2[:], gi[:], nvdec[:], Alu.subtract)
                nu = t1pool.tile([128, 512], F32, tag="nu")
                nc.vector.scalar_tensor_tensor(
                    nu[:], u2[:], 0.1, u2[:], Alu.mult, Alu.subtract)
                sl_n = s3pool.tile([128, 512], F32, tag="sl")
                nc.vector.tensor_tensor(sl_n[:], nu[:], nthr_n[:], Alu.is_le)
                nvdec_n = tpool.tile([128, 512], F32, tag="nvdec")
                nc.vector.tensor_tensor(nvdec_n[:], sbar[:], nu[:], Alu.mult)

                # --- output layer LIF, step i (reads po, lots of slack) ---
                nvodec = t1pool.tile([BL, N_OUT], F32, tag="onvdec")
                nc.vector.scalar_tensor_tensor(
                    nvodec[:], vo[:], 0.1, vo[:], Alu.mult, Alu.subtract)
                rodec = t1pool.tile([BL, N_OUT], F32, tag="ordec")
                nc.scalar.activation(rodec[:], ro[:], Act.Relu, bias=-0.5)
                nthro = t1pool.tile([BL, N_OUT], F32, tag="onthr")
                nc.scalar.activation(nthro[:], rodec[:], Act.Copy,
                                     scale=-BIG, bias=-1.0)
                so = y_acc[:, i * N_OUT:(i + 1) * N_OUT]
                nc.vector.tensor_tensor(so, nvodec[:], nthro[:], Alu.is_le)
                nc.vector.tensor_tensor(ro[:], rodec[:], so, Alu.add)
                go = t1pool.tile([BL, N_OUT], F32, tag="og")
                nc.scalar.activation(go[:], ro[:], Act.Relu,
                                     scale=-2.0, bias=1.0)
                gio = t1pool.tile([BL, N_OUT], F32, tag="ogi")
                nc.vector.tensor_tensor(gio[:], go[:], po[:BL, :], Alu.mult)
                u2o = t1pool.tile([BL, N_OUT], F32, tag="ou2")
                nc.vector.tensor_tensor(u2o[:], gio[:], nvodec[:],
                                        Alu.subtract)
                sbo = t1pool.tile([BL, N_OUT], F32, tag="osb")
                nc.vector.tensor_scalar(sbo[:], ro[:], 0.5, None, Alu.is_le)
                nc.vector.tensor_tensor(vo[:], sbo[:], u2o[:], Alu.mult)

                # rotate carried state
                sl, nvdec, rdec, nthr, ph_prev = sl_n, nvdec_n, rdec_n, nthr_n, ph

            nc.sync.dma_start(y_d.ap(), y_acc[:])

    nc.compile()
    return nc


def _get_program(t_steps):
    if t_steps not in _COMPILED:
        _COMPILED[t_steps] = _build(t_steps)
    return _COMPILED[t_steps]


def _make_in_maps(x_in, w_ih, w_hh, w_ho, t_steps):
    T = t_steps
    MT = T * BL // 128
    KI = 7
    ident = np.zeros((128, 32), np.float32)
    for q in range(4):
        ident[32 * q:32 * q + 32, :] += np.eye(32, dtype=np.float32)
    i128 = np.eye(128, dtype=np.float32)
    shared = {
        "wih": np.ascontiguousarray(w_ih, np.float32),
        "whh": np.ascontiguousarray(w_hh, np.float32),
        "who": np.ascontiguousarray(w_ho, np.float32),
        "ident": ident,
        "i128": i128,
    }
    in_maps = []
    for c in range(N_CORES):
        m = dict(shared)
        xc = np.asarray(x_in[:T, c * BL:(c + 1) * BL, :], np.float32)
        # [T,BL,784] -> [MT,4,BL,KI,112] -> [MT,KI,112,4,BL] -> [MT,KI,112,128]
        xr = xc.reshape(MT, 4, BL, KI, 112)
        m["xth"] = np.ascontiguousarray(
            xr.transpose(0, 3, 4, 1, 2).reshape(MT, KI, 112, 128))
        in_maps.append(m)
    return in_maps


def _run(x_in, w_ih, w_hh, w_ho, t_steps, trace=False):
    from concourse.bass_utils import run_bass_kernel_spmd

    nc = _get_program(t_steps)
    in_maps = _make_in_maps(x_in, w_ih, w_hh, w_ho, t_steps)
    res = run_bass_kernel_spmd(nc, in_maps, core_ids=list(range(N_CORES)),
                               trace=trace)
    parts = []
    for c in range(N_CORES):
        yc = res.results[c]["y"].reshape(BL, t_steps, N_OUT)
        parts.append(np.transpose(yc, (1, 0, 2)))
    y = np.concatenate(parts, axis=1)  # [T, B, N_OUT]
    return y, res


def kernel(x_in, w_ih, w_hh, w_ho, time):
    t_steps = int(time)
    assert t_steps == T_STEPS, f"kernel hardcoded for T={T_STEPS}, got {t_steps}"
    y, _ = _run(np.asarray(x_in), np.asarray(w_ih), np.asarray(w_hh),
                np.asarray(w_ho), t_steps)
    return y
